# revision 1
# baseline (speedup 1.0000x reference)
"""Trainium2 Bass kernel for nn_ExpectedKLDivergence.

Data-parallel over batch across 8 cores. The pairwise expected-KL term is
algebraically reduced (verified vs f64) to

    div[s] = P[s-1]*A[s] - c2*Q[s-1]*Q[s]          for 1 <= s < len
    A = p0*(ln p0 - c1) + p1*(ln p1 - c1),  P = p0+p1,  Q = p0-p1
    c1 = (ln b + ln(1-b))/2,  c2 = (ln b - ln(1-b))/2

The mask is a per-row prefix, so the host packs only the valid prefixes of
each row into a dense [128, W] stream per core (rows balanced across cores by
total valid length). A single eps=1e-6 separator element between rows makes
every cross-row pair term vanish through the multiplications (eps*ln(eps) ~
1e-28), so the device kernel needs no masking at all: the scalar engine takes
ln(x*e^-c1), the vector engine forms A/P/Q and the two shifted products, and
the tensor engine reduces them into PSUM with a ones-vector. The first-step
alpha-prior terms are computed from a tiny side input. Host combines the
per-core partials (exact c2 applied in f64) and divides by B.
"""

import numpy as np

import concourse.bacc as bacc
import concourse.mybir as mybir
import concourse.tile as tile
from concourse.bass_utils import run_bass_kernel_spmd

ALPHA = 0.1
BETA = 0.9
B, S = 512, 32768
NCORES = 8
P = 128                      # partitions
N = 2048                     # columns per tile
MM = 512                     # matmul free-dim chunk (one PSUM bank)
EPS = 1e-6                   # row separator / padding value (ln stays in a
                             # comfortable ACT range; joint terms ~1e-9 rel)

C1 = float((np.log(BETA) + np.log(1.0 - BETA)) / 2.0)
C2 = float((np.log(BETA) - np.log(1.0 - BETA)) / 2.0)
ESC = float(np.exp(-C1))     # Ln(x*ESC) = ln(x) - C1

OFFLOAD = 0                  # 0: all DVE; 1: P/Q adds on gpsimd; 2: +r2
_BUILT: dict = {}            # width -> compiled Bacc module


def _build(width: int, reps: int = 1, offload: int = 0, iob: int = 4, wkb: int = 2, lcb: int = 2, tn: int = 0):
    f32 = mybir.dt.float32
    bf = mybir.dt.bfloat16
    Ln = mybir.ActivationFunctionType.Ln
    add = mybir.AluOpType.add
    assert width % MM == 0
    TN = tn or N
    sizes = [TN] * (width // TN)
    if width % TN:
        sizes.append(width % TN)
    NT = len(sizes)
    starts = [sum(sizes[:i]) for i in range(NT)]

    nc = bacc.Bacc()
    p0d = nc.dram_tensor("p0", [P, width + 2], f32, kind="ExternalInput")
    p1d = nc.dram_tensor("p1", [P, width + 2], f32, kind="ExternalInput")
    f0d = nc.dram_tensor("f0", [P, 2], f32, kind="ExternalInput")
    outd = nc.dram_tensor("acc", [P, 4], f32, kind="ExternalOutput")

    with tile.TileContext(nc) as tc:
        with (
            tc.tile_pool(name="io", bufs=iob) as io,
            tc.tile_pool(name="lcp", bufs=lcb) as lcp,
            tc.tile_pool(name="wk", bufs=wkb) as wk,
            tc.tile_pool(name="cs", bufs=1) as cs,
            tc.tile_pool(name="psp", bufs=1, space="PSUM") as psp,
        ):
            ones = cs.tile([P, 1], bf, tag="ones")
            nc.gpsimd.memset(ones[:], 1.0)
            ps1 = psp.tile([1, MM], f32, tag="ps1")
            ps2 = psp.tile([1, MM], f32, tag="ps2")
            acc3 = cs.tile([P, 1], f32, tag="acc3")

            from contextlib import nullcontext
            loop_ctx = tc.For_i(0, reps, 1) if reps > 1 else nullcontext()
            with loop_ctx:
              for k in range(NT):
                NK = sizes[k]
                W = NK + 2
                x0 = io.tile([P, W], bf, tag="x0")
                nc.gpsimd.dma_start(x0[:], p0d[:, starts[k] : starts[k] + W])
                x1 = io.tile([P, W], bf, tag="x1")
                nc.gpsimd.dma_start(x1[:], p1d[:, starts[k] : starts[k] + W])

                lc0 = lcp.tile([P, W], bf, tag="lc0")
                nc.scalar.activation(lc0[:], x0[:], Ln, scale=ESC)
                lc1 = lcp.tile([P, W], bf, tag="lc1")
                nc.scalar.activation(lc1[:], x1[:], Ln, scale=ESC)

                af0 = wk.tile([P, W], bf, tag="af0")
                nc.vector.tensor_mul(af0[:], x0[:], lc0[:])
                af1 = wk.tile([P, W], bf, tag="af1")
                nc.vector.tensor_mul(af1[:], x1[:], lc1[:])
                a = wk.tile([P, W], bf, tag="a")
                nc.vector.tensor_add(a[:], af0[:], af1[:])

                ve_pq = nc.gpsimd if offload >= 1 else nc.vector
                pt = wk.tile([P, W], bf, tag="pt")
                ve_pq.tensor_add(pt[:], x0[:], x1[:])
                qt = wk.tile([P, W], bf, tag="qt")
                ve_pq.tensor_sub(qt[:], x0[:], x1[:])

                r1 = wk.tile([P, NK], bf, tag="af0")
                nc.vector.tensor_mul(r1[:], pt[:, 0:NK], a[:, 1 : NK + 1])
                r2 = wk.tile([P, NK], bf, tag="af1")
                ve_r2 = nc.gpsimd if offload >= 2 else nc.vector
                ve_r2.tensor_mul(r2[:], qt[:, 0:NK], qt[:, 1 : NK + 1])

                # partition+tile reduction on the tensor engine: every 512-col
                # chunk of r1/r2 accumulates into a single PSUM row
                for c in range(NK // MM):
                    nc.tensor.matmul(
                        ps1[:],
                        ones[:],
                        r1[:, c * MM : (c + 1) * MM],
                        start=(k == 0 and c == 0),
                        stop=(k == NT - 1 and c == NK // MM - 1),
                    )
                    nc.tensor.matmul(
                        ps2[:],
                        ones[:],
                        r2[:, c * MM : (c + 1) * MM],
                        start=(k == 0 and c == 0),
                        stop=(k == NT - 1 and c == NK // MM - 1),
                    )

            # first-timestep alpha-prior terms from the packed side input
            t0 = cs.tile([P, 2], bf, tag="t0")
            nc.gpsimd.dma_start(t0[:], f0d[:])
            l0 = cs.tile([P, 2], bf, tag="l0")
            nc.scalar.activation(l0[:, 0:1], t0[:, 0:1], Ln, scale=1.0 / (1.0 - ALPHA))
            nc.scalar.activation(l0[:, 1:2], t0[:, 1:2], Ln, scale=1.0 / ALPHA)
            e3 = cs.tile([P, 2], f32, tag="e3")
            nc.vector.tensor_mul(e3[:], t0[:], l0[:])
            nc.vector.tensor_reduce(acc3[:], e3[:], mybir.AxisListType.X, add)

            outsb = cs.tile([P, 4], f32, tag="outsb")
            nc.gpsimd.memset(outsb[:], 0.0)
            ps1_sb = cs.tile([1, MM], f32, tag="pss1")
            nc.vector.tensor_copy(ps1_sb[:], ps1[:])
            ps2_sb = cs.tile([1, MM], f32, tag="pss2")
            nc.vector.tensor_copy(ps2_sb[:], ps2[:])
            nc.vector.tensor_reduce(outsb[0:1, 0:1], ps1_sb[:], mybir.AxisListType.X, add)
            nc.vector.tensor_reduce(outsb[0:1, 1:2], ps2_sb[:], mybir.AxisListType.X, add)
            nc.vector.tensor_copy(outsb[:, 2:3], acc3[:])
            nc.sync.dma_start(outd[:], outsb[:])
    nc.compile()
    return nc


def _assign_rows(lengths):
    """Greedy LPT balance of rows across cores by packed size (len+1)."""
    order = np.argsort(-lengths)
    loads = np.zeros(NCORES, np.int64)
    rows = [[] for _ in range(NCORES)]
    for r in order:
        c = int(np.argmin(loads))
        rows[c].append(int(r))
        loads[c] += int(lengths[r]) + 1
    return rows, loads


def _prep_core(p0, p1, lengths, rows, width):
    """Pack valid prefixes of `rows` into [P, width+2] planes + alpha input.

    width = NT*N. Layout: col 0 is the halo (previous flat element), cols
    1..width hold the packed stream, last col is lookahead pad.
    """
    maps = {}
    for name, plane in (("p0", p0), ("p1", p1)):
        flat = np.full(P * width, EPS, np.float32)
        pos = 0
        for r in rows:
            L = int(lengths[r])
            flat[pos : pos + L] = plane[r, :L]
            pos += L + 1                      # eps separator
        arr = np.empty((P, width + 2), np.float32)
        arr[:, 1 : width + 1] = flat.reshape(P, width)
        arr[0, 0] = EPS                       # virtual past for first row
        arr[1:, 0] = arr[:-1, width]          # halo: previous flat element
        arr[:, width + 1] = EPS               # lookahead pad (never a current)
        maps[name] = arr
    f0 = np.empty((P, 2), np.float32)
    f0[:, 0] = 1.0 - ALPHA                    # pad rows contribute exactly 0
    f0[:, 1] = ALPHA
    nr = len(rows)
    f0[:nr, 0] = p0[rows, 0]
    f0[:nr, 1] = p1[rows, 0]
    maps["f0"] = f0
    return maps


def kernel(posterior, length):
    post = np.asarray(posterior, dtype=np.float32)
    ln = np.asarray(length).astype(np.int64)
    assert post.shape == (B, S, 2), post.shape
    lengths = np.clip(ln, 1, S)

    p0 = np.ascontiguousarray(post[..., 0])
    p1 = np.ascontiguousarray(post[..., 1])
    rows, loads = _assign_rows(lengths)
    # common packed width per partition, rounded up to MM granularity
    wmax = int(np.ceil(loads.max() / P))
    width = max(MM, -(-wmax // MM) * MM)

    in_maps = [
        _prep_core(p0, p1, lengths, rows[c], width) for c in range(NCORES)
    ]

    if width not in _BUILT:
        _BUILT[width] = _build(width, offload=OFFLOAD)
    res = run_bass_kernel_spmd(_BUILT[width], in_maps, core_ids=list(range(NCORES)))

    total = np.float64(0.0)
    for c, r in enumerate(res.results):
        acc = np.asarray(r["acc"], np.float64)
        total += acc[0, 0] - C2 * acc[0, 1] + acc[: len(rows[c]), 2].sum()
    return np.float32(total / B)



# revision 13
# speedup vs baseline: 2.1515x; 2.1515x over previous
"""Trainium2 Bass kernel for nn_ExpectedKLDivergence.

Data-parallel over batch across 8 cores. The pairwise expected-KL term is
algebraically reduced (verified vs f64) to

    div[s] = P[s-1]*A[s] - c2*Q[s-1]*Q[s]          for 1 <= s < len
    A = p0*(ln p0 - c1) + p1*(ln p1 - c1),  P = p0+p1,  Q = p0-p1
    c1 = (ln b + ln(1-b))/2,  c2 = (ln b - ln(1-b))/2

The mask is a per-row prefix, so the host packs only the valid prefixes of
each row into a dense bf16 [128, W] stream per core (rows balanced across
cores by total valid length), with a single eps separator element between
rows so every cross-row pair term vanishes through the multiplications.

Device pipeline per tile: ACT takes ln(x*e^-c1); DVE forms af0=x0*lc0,
af1=x1*lc1 (halves of one [P,2,W] tile), pt=x0+x1, qt=x0-x1. The shifted
dot-products sum_s pt[s-1]*(af0+af1)[s] and sum_s qt[s-1]*qt[s] run on the
tensor engine as accumulated Gram blocks: for each 128-col window,
matmul(stationary=plane[:,c:c+128], moving=other[:,c+1:c+129]) accumulates
into a PSUM [128,128(,x2)] block whose DIAGONAL holds the per-column
partition sums; everything accumulates in PSUM over the whole stream and
the diagonal is extracted once at the end with an eye-mask fused
multiply-reduce. The first-step alpha-prior terms come from a tiny side
input. Host combines per-core partials (exact c2 applied in f64) and
divides by B.
"""

import numpy as np
import ml_dtypes

import concourse.bacc as bacc
import concourse.mybir as mybir
import concourse.tile as tile
from concourse.bass_utils import run_bass_kernel_spmd

ALPHA = 0.1
BETA = 0.9
B, S = 512, 32768
NCORES = 8
P = 128                      # partitions
MM = 128                     # width granularity (Gram block size)
TN = 1664                    # columns per tile (multiple of 128)
EPS = 1e-6                   # row separator / padding value

C1 = float((np.log(BETA) + np.log(1.0 - BETA)) / 2.0)
C2 = float((np.log(BETA) - np.log(1.0 - BETA)) / 2.0)
ESC = float(np.exp(-C1))     # Ln(x*ESC) = ln(x) - C1

QT_GP_COLS = 0               # cols of qt computed on gpsimd instead of DVE
_BUILT: dict = {}            # width -> compiled Bacc module

BF16 = ml_dtypes.bfloat16


def _build(width: int, reps: int = 1, tn: int = TN, iob: int = 4, wkb: int = 2,
           lcb: int = 2, qt_gp: int = QT_GP_COLS, dma_eng: str = "sync"):
    f32 = mybir.dt.float32
    bf = mybir.dt.bfloat16
    Ln = mybir.ActivationFunctionType.Ln
    add = mybir.AluOpType.add
    mult = mybir.AluOpType.mult
    assert width % MM == 0 and tn % MM == 0
    sizes = [tn] * (width // tn)
    if width % tn:
        sizes.append(width % tn)
    NT = len(sizes)
    starts = [sum(sizes[:i]) for i in range(NT)]

    nc = bacc.Bacc()
    p0d = nc.dram_tensor("p0", [P, width + 2], bf, kind="ExternalInput")
    p1d = nc.dram_tensor("p1", [P, width + 2], bf, kind="ExternalInput")
    eyed = nc.dram_tensor("eye", [P, 256], bf, kind="ExternalInput")
    f0d = nc.dram_tensor("f0", [P, 2], bf, kind="ExternalInput")
    outd = nc.dram_tensor("acc", [P, 4], f32, kind="ExternalOutput")

    dmae = {"sync": nc.sync, "gpsimd": nc.gpsimd, "vector": nc.vector,
            "scalar": nc.scalar}[dma_eng]

    with tile.TileContext(nc) as tc:
        with (
            tc.tile_pool(name="io", bufs=iob) as io,
            tc.tile_pool(name="lcp", bufs=lcb) as lcp,
            tc.tile_pool(name="wk", bufs=wkb) as wk,
            tc.tile_pool(name="cs", bufs=1) as cs,
            tc.tile_pool(name="psp", bufs=1, space="PSUM") as psp,
        ):
            eye = cs.tile([P, 256], bf, tag="eye")
            nc.gpsimd.dma_start(eye[:], eyed[:])
            ps1a = psp.tile([P, 128], f32, tag="ps1a")
            ps1b = psp.tile([P, 128], f32, tag="ps1b")
            ps2 = psp.tile([P, 128], f32, tag="ps2")
            acc3 = cs.tile([P, 1], f32, tag="acc3")

            from contextlib import nullcontext
            loop_ctx = tc.For_i(0, reps, 1) if reps > 1 else nullcontext()
            with loop_ctx:
              for k in range(NT):
                NK = sizes[k]
                W = NK + 2
                x0 = io.tile([P, W], bf, tag="x0")
                dmae.dma_start(x0[:], p0d[:, starts[k] : starts[k] + W])
                x1 = io.tile([P, W], bf, tag="x1")
                dmae.dma_start(x1[:], p1d[:, starts[k] : starts[k] + W])

                lc0 = lcp.tile([P, W], bf, tag="lc0")
                nc.scalar.activation(lc0[:], x0[:], Ln, scale=ESC)
                lc1 = lcp.tile([P, W], bf, tag="lc1")
                nc.scalar.activation(lc1[:], x1[:], Ln, scale=ESC)

                af0 = wk.tile([P, W], bf, tag="af0")
                nc.vector.tensor_mul(af0[:], x0[:], lc0[:])
                af1 = wk.tile([P, W], bf, tag="af1")
                nc.vector.tensor_mul(af1[:], x1[:], lc1[:])
                pt = wk.tile([P, W], bf, tag="pt")
                nc.vector.tensor_add(pt[:], x0[:], x1[:])
                qt = wk.tile([P, W], bf, tag="qt")
                if qt_gp > 0:
                    sp = W - qt_gp
                    nc.vector.tensor_sub(qt[:, 0:sp], x0[:, 0:sp], x1[:, 0:sp])
                    nc.gpsimd.tensor_sub(qt[:, sp:W], x0[:, sp:W], x1[:, sp:W])
                else:
                    nc.vector.tensor_sub(qt[:], x0[:], x1[:])

                for c in range(NK // MM):
                    st = (k == 0 and c == 0)
                    sp_ = (k == NT - 1 and c == NK // MM - 1)
                    nc.tensor.matmul(
                        ps1a[:],
                        pt[:, c * MM : c * MM + MM],
                        af0[:, c * MM + 1 : c * MM + MM + 1],
                        start=st,
                        stop=sp_,
                    )
                    nc.tensor.matmul(
                        ps1b[:],
                        pt[:, c * MM : c * MM + MM],
                        af1[:, c * MM + 1 : c * MM + MM + 1],
                        start=st,
                        stop=sp_,
                    )
                    nc.tensor.matmul(
                        ps2[:],
                        qt[:, c * MM : c * MM + MM],
                        qt[:, c * MM + 1 : c * MM + MM + 1],
                        start=st,
                        stop=sp_,
                    )

            # diagonal extraction: d[p] = sum_j ps[p,j]*eye[p,j]
            d1 = cs.tile([P, 1], f32, tag="d1")
            d1b = cs.tile([P, 1], f32, tag="d1b")
            d2 = cs.tile([P, 1], f32, tag="d2")
            ey = eye[:, 0:128]
            sc1 = cs.tile([P, 128], f32, tag="sc1")
            nc.vector.tensor_mul(sc1[:], ps1a[:], ey)
            nc.vector.tensor_reduce(d1[:], sc1[:], mybir.AxisListType.X, add)
            sc1b = cs.tile([P, 128], f32, tag="sc1b")
            nc.vector.tensor_mul(sc1b[:], ps1b[:], ey)
            nc.vector.tensor_reduce(d1b[:], sc1b[:], mybir.AxisListType.X, add)
            sc2 = cs.tile([P, 128], f32, tag="sc2")
            nc.vector.tensor_mul(sc2[:], ps2[:], ey)
            nc.vector.tensor_reduce(d2[:], sc2[:], mybir.AxisListType.X, add)
            nc.vector.tensor_add(d1[:], d1[:], d1b[:])

            # first-timestep alpha-prior terms from the packed side input
            t0 = cs.tile([P, 2], bf, tag="t0")
            nc.gpsimd.dma_start(t0[:], f0d[:])
            l0 = cs.tile([P, 2], bf, tag="l0")
            nc.scalar.activation(l0[:, 0:1], t0[:, 0:1], Ln, scale=1.0 / (1.0 - ALPHA))
            nc.scalar.activation(l0[:, 1:2], t0[:, 1:2], Ln, scale=1.0 / ALPHA)
            e3 = cs.tile([P, 2], f32, tag="e3")
            nc.vector.tensor_mul(e3[:], t0[:], l0[:])
            nc.vector.tensor_reduce(acc3[:], e3[:], mybir.AxisListType.X, add)

            outsb = cs.tile([P, 4], f32, tag="outsb")
            nc.gpsimd.memset(outsb[:], 0.0)
            nc.vector.tensor_copy(outsb[:, 0:1], d1[:])
            nc.vector.tensor_copy(outsb[:, 1:2], d2[:])
            nc.vector.tensor_copy(outsb[:, 2:3], acc3[:])
            nc.sync.dma_start(outd[:], outsb[:])
    nc.compile()
    return nc


def _assign_rows(lengths):
    """Greedy LPT balance of rows across cores by packed size (len+1)."""
    order = np.argsort(-lengths)
    loads = np.zeros(NCORES, np.int64)
    rows = [[] for _ in range(NCORES)]
    for r in order:
        c = int(np.argmin(loads))
        rows[c].append(int(r))
        loads[c] += int(lengths[r]) + 1
    return rows, loads


def _prep_core(p0, p1, lengths, rows, width):
    """Pack valid prefixes of `rows` into bf16 [P, width+2] planes + consts.

    Layout: col 0 is the halo (previous flat element), cols 1..width hold
    the packed stream, last col is lookahead pad.
    """
    maps = {}
    for name, plane in (("p0", p0), ("p1", p1)):
        flat = np.full(P * width, EPS, np.float32)
        pos = 0
        for r in rows:
            L = int(lengths[r])
            flat[pos : pos + L] = plane[r, :L]
            pos += L + 1                      # eps separator
        arr = np.empty((P, width + 2), np.float32)
        arr[:, 1 : width + 1] = flat.reshape(P, width)
        arr[0, 0] = EPS                       # virtual past for first row
        arr[1:, 0] = arr[:-1, width]          # halo: previous flat element
        arr[:, width + 1] = EPS               # lookahead pad (never a current)
        maps[name] = arr.astype(BF16)
    eye = np.zeros((P, 256), np.float32)
    idx = np.arange(P)
    eye[idx, idx] = 1.0
    eye[idx, 128 + idx] = 1.0
    maps["eye"] = eye.astype(BF16)
    f0 = np.empty((P, 2), np.float32)
    f0[:, 0] = 1.0 - ALPHA                    # pad rows contribute exactly 0
    f0[:, 1] = ALPHA
    nr = len(rows)
    f0[:nr, 0] = p0[rows, 0]
    f0[:nr, 1] = p1[rows, 0]
    maps["f0"] = f0.astype(BF16)
    return maps


def kernel(posterior, length):
    post = np.asarray(posterior, dtype=np.float32)
    ln = np.asarray(length).astype(np.int64)
    assert post.shape == (B, S, 2), post.shape
    lengths = np.clip(ln, 1, S)

    p0 = np.ascontiguousarray(post[..., 0])
    p1 = np.ascontiguousarray(post[..., 1])
    rows, loads = _assign_rows(lengths)
    # common packed width per partition, rounded up to MM granularity
    wmax = int(np.ceil(loads.max() / P))
    width = max(MM, -(-wmax // MM) * MM)

    in_maps = [
        _prep_core(p0, p1, lengths, rows[c], width) for c in range(NCORES)
    ]

    if width not in _BUILT:
        _BUILT[width] = _build(width)
    res = run_bass_kernel_spmd(_BUILT[width], in_maps, core_ids=list(range(NCORES)))

    total = np.float64(0.0)
    for c, r in enumerate(res.results):
        acc = np.asarray(r["acc"], np.float64)
        total += acc[:, 0].sum() - C2 * acc[:, 1].sum() + acc[: len(rows[c]), 2].sum()
    return np.float32(total / B)
